# revision 1
# baseline (speedup 1.0000x reference)
"""Deformable conv (nn_DeformConv_31267361915085) Trainium2 Bass kernel.

Sharding: data-parallel over (batch, H-half): core n handles batch n//2,
output rows [28*(n%2), 28*(n%2)+28). Weights replicated. SPMD: one program;
per-core input slabs are pre-shifted on host so the program is core-agnostic.

Per-core pipeline (on device):
  1. offset conv: 9 taps x 2 c-chunks of fp32r matmuls, PSUM-accumulated
  2. PE-transpose offsets to pixel-on-partition layout; fp32 coordinate and
     bilinear-weight math on DVE (floor via int cast + compare fixup)
  3. dma_gather of 2x2 "quad" corner vectors (bf16, 2KB elements) from a
     zero-padded channels-last quad table in DRAM; out-of-image corners
     gather zeros, which reproduces the reference's OOB masking exactly
  4. bilinear lerp: per (block, tap) fused scalar_tensor_tensor ops with
     per-partition (= per-pixel) corner weights
  5. PE-transpose patches to [ck, pixel] layout, 18-chunk bf16 matmul with
     the main conv weights, PSUM accumulate, DMA out.
"""

import sys

if "/opt/trn_rl_repo" not in sys.path:
    sys.path.insert(0, "/opt/trn_rl_repo")

import contextlib

import numpy as np
import ml_dtypes

import concourse.bass as bass
import concourse.tile as tile
from concourse import bacc, mybir
from concourse.bass_utils import run_bass_kernel_spmd
from concourse.masks import make_identity

F32 = mybir.dt.float32
F32R = mybir.dt.float32r
BF16 = mybir.dt.bfloat16
I16 = mybir.dt.int16
I32 = mybir.dt.int32
AL = mybir.AluOpType

# problem dims
B, CIN, H, W = 4, 256, 56, 56
COUT = 256
KK = 9
MARG = 8                # gather pad margin (covers |offset| <= ~6)
HQ = WQ = H + 2 * MARG  # 72: quad-table grid
NQ = HQ * WQ            # 5184 quad rows
NROWS = 28              # output rows per core
NPIX = NROWS * W        # 1568
BLK = 112               # pixels per block (2 output rows)
NBLK = NPIX // BLK      # 14
SLOT = 128              # gather slots per (tap, block): 112 real + 16 pad
NIDX = KK * SLOT        # 1152 gather indices per block
NSL = 448               # main matmul N-slice
NSLOT = NBLK * SLOT     # 1792 slot-columns

_CACHE = {}


def _ap(base, offset_elems, dims):
    """AP with explicit free dims on top of a tile's base AP."""
    return bass.AP(
        tensor=base.tensor, offset=base.offset + offset_elems, ap=[base.ap[0]] + dims
    )


def _fold_idx(nc, idxT16, idxw):
    """[126 (bb,k), 112 pix] int16 -> SWDGE wrapped [128, NBLK, 72] int16.

    idxw[q + 16r, bb, k*8 + t] = idxT16[bb*9 + k, q*7 + t]  (t<7; t==7 stays 0)

    idxT16 columns are pre-permuted to wrap order (col q*7+t = pixel t*16+q),
    so every fold run is contiguous.
    """
    it = idxT16[:, :]
    iw = idxw[:, :, :]
    ppw = iw.ap[0][0]
    for q in range(16):
        src = bass.AP(
            tensor=it.tensor,
            offset=it.offset + q * 7,
            ap=[it.ap[0], [1, 7]],
        )
        dst = bass.AP(
            tensor=iw.tensor,
            offset=iw.offset + q * ppw,
            ap=[[ppw, 1], [8, 126], [1, 7]],
        )
        nc.sync.dma_start(out=dst, in_=src)
    rep = NBLK * 72
    for r in range(1, 8):
        src = bass.AP(tensor=iw.tensor, offset=iw.offset, ap=[[ppw, 16], [1, rep]])
        dst = bass.AP(
            tensor=iw.tensor,
            offset=iw.offset + 16 * r * ppw,
            ap=[[ppw, 16], [1, rep]],
        )
        nc.sync.dma_start(out=dst, in_=src)


def build_nc():
    nc = bacc.Bacc(None, target_bir_lowering=False)

    xcf_d = nc.dram_tensor("xcf", [128, 2, 30 * 58], BF16, kind="ExternalInput")
    xq_d = nc.dram_tensor("xq", [NQ, 1024], BF16, kind="ExternalInput")
    woff_d = nc.dram_tensor("woff", [128, 2, KK, 18], BF16, kind="ExternalInput")
    boff_d = nc.dram_tensor("boff", [18, 1], F32, kind="ExternalInput")
    wm_d = nc.dram_tensor("wm", [128, KK, 2, 2, 128], BF16, kind="ExternalInput")
    out_d = nc.dram_tensor("out", [128, 2, NSLOT], F32, kind="ExternalOutput")

    with tile.TileContext(nc) as tc, contextlib.ExitStack() as ctx:
        singles = ctx.enter_context(tc.tile_pool(name="singles", bufs=1))
        coords = ctx.enter_context(tc.tile_pool(name="coords", bufs=1))

        # ---- load constants / weights / activations ----
        xcf = singles.tile([128, 2, 30 * 58], BF16)
        nc.sync.dma_start(out=xcf[:, :, :], in_=xcf_d[:, :, :])
        woff = singles.tile([128, 2, KK, 18], BF16)
        nc.sync.dma_start(out=woff[:, :, :, :], in_=woff_d[:, :, :, :])
        boff = singles.tile([18, 1], F32)
        nc.sync.dma_start(out=boff[:, :], in_=boff_d[:, :])
        wm = singles.tile([128, KK, 2, 2, 128], BF16)
        nc.sync.dma_start(out=wm[:, :, :, :, :], in_=wm_d[:, :, :, :, :])

        ident_f = singles.tile([128, 128], F32)
        make_identity(nc, ident_f[:, :])
        ident_b = singles.tile([128, 128], BF16)
        nc.vector.tensor_copy(out=ident_b[:, :], in_=ident_f[:, :])

        # iota-derived planes (core-independent)
        it_i = coords.tile([128, 1], I32)
        nc.gpsimd.iota(it_i[:, :], pattern=[[0, 1]], base=0, channel_multiplier=1)
        p_f = coords.tile([128, 1], F32)
        nc.vector.tensor_copy(out=p_f[:, :], in_=it_i[:, :])
        pge = coords.tile([128, 1], F32)  # 1.0 if partition >= 56
        nc.vector.tensor_scalar(
            out=pge[:, :], in0=p_f[:, :], scalar1=56.0, scalar2=None, op0=AL.is_ge
        )
        jx = coords.tile([128, 1], F32)  # j = p - 56*(p>=56)
        nc.vector.scalar_tensor_tensor(
            out=jx[:, :], in0=pge[:, :], scalar=-56.0, in1=p_f[:, :],
            op0=AL.mult, op1=AL.add,
        )
        bb2_i = coords.tile([128, NBLK], I32)
        nc.gpsimd.iota(bb2_i[:, :], pattern=[[2, NBLK]], base=0, channel_multiplier=0)
        iy2 = coords.tile([128, NBLK], F32)  # block-local row: 2*bb + (p>=56)
        nc.vector.tensor_copy(out=iy2[:, :], in_=bb2_i[:, :])
        nc.vector.tensor_tensor(
            out=iy2[:, :], in0=iy2[:, :], in1=_ap(pge[:], 0, [[0, NBLK]]), op=AL.add
        )
        kyM_i = coords.tile([128, KK], I32)
        nc.gpsimd.iota(
            kyM_i[:, :], pattern=[[1, 3], [0, 3]], base=MARG - 1, channel_multiplier=0
        )
        kyM = coords.tile([128, KK], F32)
        nc.vector.tensor_copy(out=kyM[:, :], in_=kyM_i[:, :])
        kxM_i = coords.tile([128, KK], I32)
        nc.gpsimd.iota(
            kxM_i[:, :], pattern=[[0, 3], [1, 3]], base=MARG - 1, channel_multiplier=0
        )
        kxM = coords.tile([128, KK], F32)
        nc.vector.tensor_copy(out=kxM[:, :], in_=kxM_i[:, :])

        # ---- offset conv ----
        off_sb = coords.tile([18, 4 * 392], F32)
        with tc.tile_pool(name="po", bufs=2, space="PSUM") as po:
            for ns in range(4):
                ps_o = po.tile([18, 392], F32)
                for kc in range(18):
                    k, ch = divmod(kc, 2)
                    ky, kx = divmod(k, 3)
                    rhs = _ap(
                        xcf[:, :, :],
                        ch * 1740 + (ns * 7 + ky) * 58 + kx,
                        [[58, 7], [1, 56]],
                    )
                    nc.tensor.matmul(
                        ps_o[:, :],
                        woff[:, ch, k, :],
                        rhs,
                        start=(kc == 0),
                        stop=(kc == 17),
                    )
                nc.vector.tensor_scalar(
                    out=off_sb[:, ns * 392 : (ns + 1) * 392],
                    in0=ps_o[:, :],
                    scalar1=boff[:, 0:1],
                    scalar2=None,
                    op0=AL.add,
                )

        # ---- transpose offsets to pixel-on-partition ----
        offT = coords.tile([128, NBLK, 18], F32)
        nc.vector.memset(offT[:, :, :], 0.0)
        with tc.tile_pool(name="pot", bufs=2, space="PSUM") as pot:
            for bb in range(NBLK):
                ps_t = pot.tile([112, 18], F32)
                nc.tensor.transpose(
                    ps_t[:, :],
                    off_sb[:18, bb * BLK : (bb + 1) * BLK],
                    ident_f[:18, :18],
                )
                nc.vector.tensor_copy(out=offT[:112, bb, :], in_=ps_t[:, :])

        # ---- coordinate + weight math (fp32 [128, NBLK, 9] planes) ----
        _pc = [0]

        def plane():
            _pc[0] += 1
            return coords.tile([128, NBLK, KK], F32, name=f"cplane{_pc[0]}")

        def keep(t):  # tiles that stay live through the block loop
            return t

        dy = _ap(offT[:], 0, [[18, NBLK], [2, KK]])
        dx = _ap(offT[:], 1, [[18, NBLK], [2, KK]])
        iy_b = _ap(iy2[:], 0, [[1, NBLK], [0, KK]])
        jx_b = _ap(jx[:], 0, [[0, NBLK], [0, KK]])
        kyM_b = _ap(kyM[:], 0, [[0, NBLK], [1, KK]])
        kxM_b = _ap(kxM[:], 0, [[0, NBLK], [1, KK]])

        pym = coords.tile([128, NBLK, KK], F32)
        pxm = coords.tile([128, NBLK, KK], F32)
        nc.vector.tensor_tensor(out=pym[:, :, :], in0=dy, in1=iy_b, op=AL.add)
        nc.vector.tensor_tensor(out=pym[:, :, :], in0=pym[:, :, :], in1=kyM_b, op=AL.add)
        nc.vector.tensor_tensor(out=pxm[:, :, :], in0=dx, in1=jx_b, op=AL.add)
        nc.vector.tensor_tensor(out=pxm[:, :, :], in0=pxm[:, :, :], in1=kxM_b, op=AL.add)

        def floor_of(src):
            ci = coords.tile([128, NBLK, KK], I32, name=f"ci{_pc[0]}")
            nc.vector.tensor_copy(out=ci[:, :, :], in_=src[:, :, :])
            cf = plane()
            nc.vector.tensor_copy(out=cf[:, :, :], in_=ci[:, :, :])
            gt = plane()
            nc.vector.tensor_tensor(
                out=gt[:, :, :], in0=cf[:, :, :], in1=src[:, :, :], op=AL.is_gt
            )
            nc.vector.tensor_tensor(
                out=cf[:, :, :], in0=cf[:, :, :], in1=gt[:, :, :], op=AL.subtract
            )
            return cf

        y0 = floor_of(pym)
        x0 = floor_of(pxm)
        ty = coords.tile([128, NBLK, KK], F32)
        tx = coords.tile([128, NBLK, KK], F32)
        nc.vector.tensor_tensor(
            out=ty[:, :, :], in0=pym[:, :, :], in1=y0[:, :, :], op=AL.subtract
        )
        nc.vector.tensor_tensor(
            out=tx[:, :, :], in0=pxm[:, :, :], in1=x0[:, :, :], op=AL.subtract
        )

        # clamp into quad table (clamped region is zero-padded -> exact)
        y0c, x0c = plane(), plane()
        nc.vector.tensor_scalar(
            out=y0c[:, :, :], in0=y0[:, :, :], scalar1=0.0, scalar2=float(HQ - 1),
            op0=AL.max, op1=AL.min,
        )
        nc.vector.tensor_scalar(
            out=x0c[:, :, :], in0=x0[:, :, :], scalar1=0.0, scalar2=float(WQ - 1),
            op0=AL.max, op1=AL.min,
        )
        idxf = plane()
        nc.vector.scalar_tensor_tensor(
            out=idxf[:, :, :], in0=y0c[:, :, :], scalar=float(WQ), in1=x0c[:, :, :],
            op0=AL.mult, op1=AL.add,
        )
        idxT16 = coords.tile([126, BLK], I16)
        with tc.tile_pool(name="pidx", bufs=1, space="PSUM") as pidx:
            ps_i = pidx.tile([126, 128], F32)
            nc.tensor.transpose(
                ps_i[:, :], _ap(idxf[:, :, :], 0, [[1, 126]]), ident_f[:, :]
            )
            # permute columns to wrap order: dst col q*7+t <- pixel t*16+q
            nc.vector.tensor_copy(
                out=idxT16[:, :], in_=_ap(ps_i[:, :], 0, [[1, 16], [16, 7]])
            )


        # ---- fold indices into SWDGE wrapped layout ----
        idxw = coords.tile([128, NBLK, 72], I16)
        nc.vector.memset(idxw[:, :, :], 0)
        _fold_idx(nc, idxT16, idxw)

        # ---- gather + lerp + transpose per block ----
        rhs_buf = singles.tile([128, KK, 2, NSLOT], BF16)
        with (
            tc.tile_pool(name="gp", bufs=2) as gp,
            tc.tile_pool(name="pp", bufs=3) as pp,
            tc.tile_pool(name="ptb", bufs=2, space="PSUM") as ptb,
        ):
            for bb in range(NBLK):
                g = gp.tile([128, KK, 1024], BF16)
                nc.gpsimd.dma_gather(
                    out_ap=g[:, :, :],
                    in_ap=xq_d[:, :],
                    idxs_ap=idxw[:, bb, :],
                    num_idxs=NIDX,
                    num_idxs_reg=NIDX,
                    elem_size=1024,
                    single_packet=False,
                )
                ps_b = ptb.tile([128, KK, 2, 128], BF16)
                for k in range(KK):
                    gk = g[:, k, 0:1024]
                    bd = _ap(gk, 256, [[512, 2], [1, 256]])
                    ac = _ap(gk, 0, [[512, 2], [1, 256]])
                    hx = pp.tile([128, 512], BF16, tag="hx", name="hx")
                    nc.vector.tensor_tensor(out=hx[:, :], in0=bd, in1=ac, op=AL.subtract)
                    nc.vector.scalar_tensor_tensor(
                        out=hx[:, :], in0=hx[:, :], scalar=tx[:, bb, k : k + 1],
                        in1=ac, op0=AL.mult, op1=AL.add,
                    )
                    pt = pp.tile([128, 256], BF16)
                    nc.vector.tensor_tensor(
                        out=pt[:, :], in0=hx[:, 256:512], in1=hx[:, 0:256],
                        op=AL.subtract,
                    )
                    nc.vector.scalar_tensor_tensor(
                        out=pt[:, :], in0=pt[:, :], scalar=ty[:, bb, k : k + 1],
                        in1=hx[:, 0:256], op0=AL.mult, op1=AL.add,
                    )
                    for ch in range(2):
                        nc.tensor.transpose(
                            ps_b[:, k, ch, :],
                            pt[:, ch * 128 : (ch + 1) * 128],
                            ident_b[:, :],
                        )
                nc.scalar.copy(
                    out=rhs_buf[:, :, :, bb * SLOT : (bb + 1) * SLOT],
                    in_=ps_b[:, :, :, :],
                )

        # ---- main conv matmul ----
        out_sb = singles.tile([128, 2, NSLOT], F32)
        with tc.tile_pool(name="pm", bufs=4, space="PSUM") as pm:
            for ot in range(2):
                pms = [pm.tile([128, NSL], F32, tag="pmtile", name="pmtile") for _ in range(4)]
                for kc in range(18):
                    k, ch = divmod(kc, 2)
                    for ns in range(4):
                        nc.tensor.matmul(
                            pms[ns][:, :],
                            wm[:, k, ch, ot, :],
                            rhs_buf[:, k, ch, ns * NSL : (ns + 1) * NSL],
                            start=(kc == 0),
                            stop=(kc == 17),
                        )
                for ns in range(4):
                    nc.scalar.copy(
                        out=out_sb[:, ot, ns * NSL : (ns + 1) * NSL],
                        in_=pms[ns][:, :],
                    )
        nc.sync.dma_start(out=out_d[:, :, :], in_=out_sb[:, :, :])

    nc.compile()
    return nc


def prep_inputs(x, w_off, b_off, w):
    """Host-side slab/layout prep. Returns list of 8 per-core input dicts."""
    x = np.asarray(x, dtype=np.float32)
    w_off = np.asarray(w_off, dtype=np.float32)
    b_off = np.asarray(b_off, dtype=np.float32)
    w = np.asarray(w, dtype=np.float32)

    woff_arr = np.ascontiguousarray(
        w_off.reshape(18, 2, 128, KK).transpose(2, 1, 3, 0)
    ).astype(ml_dtypes.bfloat16)  # [128 cl, 2 ch, 9 k, 18 o]
    boff_arr = np.ascontiguousarray(b_off.reshape(18, 1))
    wm_arr = np.ascontiguousarray(
        w.reshape(2, 128, 2, 128, KK).transpose(3, 4, 2, 0, 1)
    ).astype(ml_dtypes.bfloat16)  # [128 cl, 9 k, 2 ch, 2 ot, 128 ol]

    in_maps = []
    for core in range(8):
        b, half = divmod(core, 2)
        r0 = half * NROWS
        xb = x[b]  # [256, 56, 56]

        xp58 = np.zeros((CIN, 58, 58), np.float32)
        xp58[:, 1:57, 1:57] = xb
        xcf = np.ascontiguousarray(
            xp58[:, r0 : r0 + 30, :].reshape(2, 128, 30 * 58).transpose(1, 0, 2)
        ).astype(ml_dtypes.bfloat16)

        xp = np.zeros((HQ + 1, WQ + 1, CIN), np.float32)
        ylo = max(0, r0 - MARG)
        yhi = min(H, r0 + HQ + 1 - MARG)
        xhwc = xb.transpose(1, 2, 0)
        xp[ylo - (r0 - MARG) : yhi - (r0 - MARG), MARG : MARG + W, :] = xhwc[ylo:yhi]
        quad = np.stack(
            [xp[:-1, :-1], xp[:-1, 1:], xp[1:, :-1], xp[1:, 1:]], axis=2
        )  # [72, 72, 4, 256]
        xq = np.ascontiguousarray(quad.reshape(NQ, 4 * CIN)).astype(ml_dtypes.bfloat16)

        in_maps.append(
            {
                "xcf": xcf,
                "xq": xq,
                "woff": woff_arr,
                "boff": boff_arr,
                "wm": wm_arr,
            }
        )
    return in_maps


def unshard_output(results):
    """results: list of 8 per-core out arrays [128, 2, NSLOT] -> [B,COUT,H,W]."""
    out = np.zeros((B, COUT, H, W), np.float32)
    for core in range(8):
        b, half = divmod(core, 2)
        r0 = half * NROWS
        oc = results[core]  # [128 ol, 2 ot, 1792]
        oc = oc.reshape(128, 2, NBLK, SLOT)[:, :, :, :BLK]
        oc = oc.transpose(1, 0, 2, 3).reshape(COUT, NROWS, W)
        out[b, :, r0 : r0 + NROWS, :] = oc
    return out


def kernel(**inputs):
    nc = _CACHE.get("nc")
    if nc is None:
        nc = build_nc()
        _CACHE["nc"] = nc
    in_maps = prep_inputs(
        inputs["x"], inputs["w_off"], inputs["b_off"], inputs["w"]
    )
    res = run_bass_kernel_spmd(nc, in_maps, core_ids=list(range(8)))
    return unshard_output([r["out"] for r in res.results])



# revision 4
# speedup vs baseline: 1.0384x; 1.0384x over previous
"""Deformable conv (nn_DeformConv_31267361915085) Trainium2 Bass kernel.

Sharding: data-parallel over (batch, H-half): core n handles batch n//2,
output rows [28*(n%2), 28*(n%2)+28). Weights replicated. SPMD: one program;
per-core input slabs are pre-shifted on host so the program is core-agnostic.

Per-core pipeline (on device):
  1. offset conv: 9 taps x 2 c-chunks of fp32r matmuls, PSUM-accumulated
  2. PE-transpose offsets to pixel-on-partition layout; fp32 coordinate and
     bilinear-weight math on DVE (floor via int cast + compare fixup)
  3. dma_gather of 2x2 "quad" corner vectors (bf16, 2KB elements) from a
     zero-padded channels-last quad table in DRAM; out-of-image corners
     gather zeros, which reproduces the reference's OOB masking exactly
  4. bilinear lerp: per (block, tap) fused scalar_tensor_tensor ops with
     per-partition (= per-pixel) corner weights
  5. PE-transpose patches to [ck, pixel] layout, 18-chunk bf16 matmul with
     the main conv weights, PSUM accumulate, DMA out.
"""

import sys

if "/opt/trn_rl_repo" not in sys.path:
    sys.path.insert(0, "/opt/trn_rl_repo")

import contextlib

import numpy as np
import ml_dtypes

import concourse.bass as bass
import concourse.tile as tile
from concourse import bacc, mybir
from concourse.bass_utils import run_bass_kernel_spmd
from concourse.masks import make_identity

F32 = mybir.dt.float32
F32R = mybir.dt.float32r
BF16 = mybir.dt.bfloat16
I16 = mybir.dt.int16
I32 = mybir.dt.int32
AL = mybir.AluOpType

# problem dims
B, CIN, H, W = 4, 256, 56, 56
COUT = 256
KK = 9
MARG = 8                # gather pad margin (covers |offset| <= ~6)
HQ = WQ = H + 2 * MARG  # 72: quad-table grid
NQ = HQ * WQ            # 5184 quad rows
NROWS = 28              # output rows per core
NPIX = NROWS * W        # 1568
BLK = 112               # pixels per block (2 output rows)
NBLK = NPIX // BLK      # 14
SLOT = 128              # gather slots per (tap, block): 112 real + 16 pad
NIDX = KK * SLOT        # 1152 gather indices per block
NSL = 448               # main matmul N-slice
NSLOT = NBLK * SLOT     # 1792 slot-columns

_CACHE = {}


def _ap(base, offset_elems, dims):
    """AP with explicit free dims on top of a tile's base AP."""
    return bass.AP(
        tensor=base.tensor, offset=base.offset + offset_elems, ap=[base.ap[0]] + dims
    )


def _fold_idx(nc, idxT16, idxw):
    """[126 (bb,k), 112 pix] int16 -> SWDGE wrapped [128, NBLK, 72] int16.

    idxw[q + 16r, bb, k*8 + t] = idxT16[bb*9 + k, q*7 + t]  (t<7; t==7 stays 0)

    idxT16 columns are pre-permuted to wrap order (col q*7+t = pixel t*16+q),
    so every fold run is contiguous.
    """
    it = idxT16[:, :]
    iw = idxw[:, :, :]
    ppw = iw.ap[0][0]
    for q in range(16):
        src = bass.AP(
            tensor=it.tensor,
            offset=it.offset + q * 7,
            ap=[it.ap[0], [1, 7]],
        )
        dst = bass.AP(
            tensor=iw.tensor,
            offset=iw.offset + q * ppw,
            ap=[[ppw, 1], [8, 126], [1, 7]],
        )
        nc.sync.dma_start(out=dst, in_=src)
    rep = NBLK * 72
    for r in range(1, 8):
        src = bass.AP(tensor=iw.tensor, offset=iw.offset, ap=[[ppw, 16], [1, rep]])
        dst = bass.AP(
            tensor=iw.tensor,
            offset=iw.offset + 16 * r * ppw,
            ap=[[ppw, 16], [1, rep]],
        )
        nc.sync.dma_start(out=dst, in_=src)


def build_nc():
    nc = bacc.Bacc(None, target_bir_lowering=False, num_swdge_queues=4)

    xcf_d = nc.dram_tensor("xcf", [128, 2, 30 * 58], BF16, kind="ExternalInput")
    xq_d = nc.dram_tensor("xq", [NQ, 1024], BF16, kind="ExternalInput")
    woff_d = nc.dram_tensor("woff", [128, 2, KK, 18], BF16, kind="ExternalInput")
    boff_d = nc.dram_tensor("boff", [18, 1], F32, kind="ExternalInput")
    wm_d = nc.dram_tensor("wm", [128, KK, 2, 2, 128], BF16, kind="ExternalInput")
    out_d = nc.dram_tensor("out", [128, 2, NSLOT], F32, kind="ExternalOutput")

    with tile.TileContext(nc) as tc, contextlib.ExitStack() as ctx:
        singles = ctx.enter_context(tc.tile_pool(name="singles", bufs=1))
        coords = ctx.enter_context(tc.tile_pool(name="coords", bufs=1))

        # ---- load constants / weights / activations ----
        xcf = singles.tile([128, 2, 30 * 58], BF16)
        nc.sync.dma_start(out=xcf[:, :, :], in_=xcf_d[:, :, :])
        woff = singles.tile([128, 2, KK, 18], BF16)
        nc.sync.dma_start(out=woff[:, :, :, :], in_=woff_d[:, :, :, :])
        boff = singles.tile([18, 1], F32)
        nc.sync.dma_start(out=boff[:, :], in_=boff_d[:, :])
        wm = singles.tile([128, KK, 2, 2, 128], BF16)
        nc.sync.dma_start(out=wm[:, :, :, :, :], in_=wm_d[:, :, :, :, :])

        ident_f = singles.tile([128, 128], F32)
        make_identity(nc, ident_f[:, :])
        ident_b = singles.tile([128, 128], BF16)
        nc.vector.tensor_copy(out=ident_b[:, :], in_=ident_f[:, :])

        # iota-derived planes (core-independent)
        it_i = coords.tile([128, 1], I32)
        nc.gpsimd.iota(it_i[:, :], pattern=[[0, 1]], base=0, channel_multiplier=1)
        p_f = coords.tile([128, 1], F32)
        nc.vector.tensor_copy(out=p_f[:, :], in_=it_i[:, :])
        pge = coords.tile([128, 1], F32)  # 1.0 if partition >= 56
        nc.vector.tensor_scalar(
            out=pge[:, :], in0=p_f[:, :], scalar1=56.0, scalar2=None, op0=AL.is_ge
        )
        jx = coords.tile([128, 1], F32)  # j = p - 56*(p>=56)
        nc.vector.scalar_tensor_tensor(
            out=jx[:, :], in0=pge[:, :], scalar=-56.0, in1=p_f[:, :],
            op0=AL.mult, op1=AL.add,
        )
        bb2_i = coords.tile([128, NBLK], I32)
        nc.gpsimd.iota(bb2_i[:, :], pattern=[[2, NBLK]], base=0, channel_multiplier=0)
        iy2 = coords.tile([128, NBLK], F32)  # block-local row: 2*bb + (p>=56)
        nc.vector.tensor_copy(out=iy2[:, :], in_=bb2_i[:, :])
        nc.vector.tensor_tensor(
            out=iy2[:, :], in0=iy2[:, :], in1=_ap(pge[:], 0, [[0, NBLK]]), op=AL.add
        )
        kyM_i = coords.tile([128, KK], I32)
        nc.gpsimd.iota(
            kyM_i[:, :], pattern=[[1, 3], [0, 3]], base=MARG - 1, channel_multiplier=0
        )
        kyM = coords.tile([128, KK], F32)
        nc.vector.tensor_copy(out=kyM[:, :], in_=kyM_i[:, :])
        kxM_i = coords.tile([128, KK], I32)
        nc.gpsimd.iota(
            kxM_i[:, :], pattern=[[0, 3], [1, 3]], base=MARG - 1, channel_multiplier=0
        )
        kxM = coords.tile([128, KK], F32)
        nc.vector.tensor_copy(out=kxM[:, :], in_=kxM_i[:, :])

        # ---- offset conv ----
        off_sb = coords.tile([18, 4 * 392], F32)
        with tc.tile_pool(name="po", bufs=2, space="PSUM") as po:
            for ns in range(4):
                ps_o = po.tile([18, 392], F32)
                for kc in range(18):
                    k, ch = divmod(kc, 2)
                    ky, kx = divmod(k, 3)
                    rhs = _ap(
                        xcf[:, :, :],
                        ch * 1740 + (ns * 7 + ky) * 58 + kx,
                        [[58, 7], [1, 56]],
                    )
                    nc.tensor.matmul(
                        ps_o[:, :],
                        woff[:, ch, k, :],
                        rhs,
                        start=(kc == 0),
                        stop=(kc == 17),
                    )
                nc.vector.tensor_scalar(
                    out=off_sb[:, ns * 392 : (ns + 1) * 392],
                    in0=ps_o[:, :],
                    scalar1=boff[:, 0:1],
                    scalar2=None,
                    op0=AL.add,
                )

        # ---- transpose offsets to pixel-on-partition ----
        offT = coords.tile([128, NBLK, 18], F32)
        nc.vector.memset(offT[:, :, :], 0.0)
        with tc.tile_pool(name="pot", bufs=2, space="PSUM") as pot:
            for bb in range(NBLK):
                ps_t = pot.tile([112, 18], F32)
                nc.tensor.transpose(
                    ps_t[:, :],
                    off_sb[:18, bb * BLK : (bb + 1) * BLK],
                    ident_f[:18, :18],
                )
                nc.vector.tensor_copy(out=offT[:112, bb, :], in_=ps_t[:, :])

        # ---- coordinate + weight math (fp32 [128, NBLK, 9] planes) ----
        _pc = [0]

        def plane():
            _pc[0] += 1
            return coords.tile([128, NBLK, KK], F32, name=f"cplane{_pc[0]}")

        def keep(t):  # tiles that stay live through the block loop
            return t

        dy = _ap(offT[:], 0, [[18, NBLK], [2, KK]])
        dx = _ap(offT[:], 1, [[18, NBLK], [2, KK]])
        iy_b = _ap(iy2[:], 0, [[1, NBLK], [0, KK]])
        jx_b = _ap(jx[:], 0, [[0, NBLK], [0, KK]])
        kyM_b = _ap(kyM[:], 0, [[0, NBLK], [1, KK]])
        kxM_b = _ap(kxM[:], 0, [[0, NBLK], [1, KK]])

        pym = coords.tile([128, NBLK, KK], F32)
        pxm = coords.tile([128, NBLK, KK], F32)
        nc.vector.tensor_tensor(out=pym[:, :, :], in0=dy, in1=iy_b, op=AL.add)
        nc.vector.tensor_tensor(out=pym[:, :, :], in0=pym[:, :, :], in1=kyM_b, op=AL.add)
        nc.vector.tensor_tensor(out=pxm[:, :, :], in0=dx, in1=jx_b, op=AL.add)
        nc.vector.tensor_tensor(out=pxm[:, :, :], in0=pxm[:, :, :], in1=kxM_b, op=AL.add)

        def floor_of(src):
            ci = coords.tile([128, NBLK, KK], I32, name=f"ci{_pc[0]}")
            nc.vector.tensor_copy(out=ci[:, :, :], in_=src[:, :, :])
            cf = plane()
            nc.vector.tensor_copy(out=cf[:, :, :], in_=ci[:, :, :])
            gt = plane()
            nc.vector.tensor_tensor(
                out=gt[:, :, :], in0=cf[:, :, :], in1=src[:, :, :], op=AL.is_gt
            )
            nc.vector.tensor_tensor(
                out=cf[:, :, :], in0=cf[:, :, :], in1=gt[:, :, :], op=AL.subtract
            )
            return cf

        y0 = floor_of(pym)
        x0 = floor_of(pxm)
        ty = coords.tile([128, NBLK, KK], F32)
        tx = coords.tile([128, NBLK, KK], F32)
        nc.vector.tensor_tensor(
            out=ty[:, :, :], in0=pym[:, :, :], in1=y0[:, :, :], op=AL.subtract
        )
        nc.vector.tensor_tensor(
            out=tx[:, :, :], in0=pxm[:, :, :], in1=x0[:, :, :], op=AL.subtract
        )

        # clamp into quad table (clamped region is zero-padded -> exact)
        y0c, x0c = plane(), plane()
        nc.vector.tensor_scalar(
            out=y0c[:, :, :], in0=y0[:, :, :], scalar1=0.0, scalar2=float(HQ - 1),
            op0=AL.max, op1=AL.min,
        )
        nc.vector.tensor_scalar(
            out=x0c[:, :, :], in0=x0[:, :, :], scalar1=0.0, scalar2=float(WQ - 1),
            op0=AL.max, op1=AL.min,
        )
        idxf = plane()
        nc.vector.scalar_tensor_tensor(
            out=idxf[:, :, :], in0=y0c[:, :, :], scalar=float(WQ), in1=x0c[:, :, :],
            op0=AL.mult, op1=AL.add,
        )
        idxT16 = coords.tile([126, BLK], I16)
        with tc.tile_pool(name="pidx", bufs=1, space="PSUM") as pidx:
            ps_i = pidx.tile([126, 128], F32)
            nc.tensor.transpose(
                ps_i[:, :], _ap(idxf[:, :, :], 0, [[1, 126]]), ident_f[:, :]
            )
            # permute columns to wrap order: dst col q*7+t <- pixel t*16+q
            nc.vector.tensor_copy(
                out=idxT16[:, :], in_=_ap(ps_i[:, :], 0, [[1, 16], [16, 7]])
            )


        # ---- fold indices into SWDGE wrapped layout ----
        idxw = coords.tile([128, NBLK, 72], I16)
        nc.vector.memset(idxw[:, :, :], 0)
        _fold_idx(nc, idxT16, idxw)

        # ---- gather + lerp + transpose per block ----
        rhs_buf = singles.tile([128, KK, 2, NSLOT], BF16)
        with (
            tc.tile_pool(name="gp", bufs=4) as gp,
            tc.tile_pool(name="pp", bufs=3) as pp,
            tc.tile_pool(name="ptb", bufs=2, space="PSUM") as ptb,
        ):
            for bb in range(NBLK):
                g = gp.tile([128, KK, 1024], BF16)
                nc.gpsimd.dma_gather(
                    out_ap=g[:, :, :],
                    in_ap=xq_d[:, :],
                    idxs_ap=idxw[:, bb, :],
                    num_idxs=NIDX,
                    num_idxs_reg=NIDX,
                    elem_size=1024,
                    single_packet=False,
                    queue_num=bb % 4,
                )
                ps_b = ptb.tile([128, KK, 2, 128], BF16)
                for k in range(KK):
                    gk = g[:, k, 0:1024]
                    bd = _ap(gk, 256, [[512, 2], [1, 256]])
                    ac = _ap(gk, 0, [[512, 2], [1, 256]])
                    hx = pp.tile([128, 512], BF16, tag="hx", name="hx")
                    nc.vector.tensor_tensor(out=hx[:, :], in0=bd, in1=ac, op=AL.subtract)
                    nc.vector.scalar_tensor_tensor(
                        out=hx[:, :], in0=hx[:, :], scalar=tx[:, bb, k : k + 1],
                        in1=ac, op0=AL.mult, op1=AL.add,
                    )
                    pt = pp.tile([128, 256], BF16)
                    nc.vector.tensor_tensor(
                        out=pt[:, :], in0=hx[:, 256:512], in1=hx[:, 0:256],
                        op=AL.subtract,
                    )
                    nc.vector.scalar_tensor_tensor(
                        out=pt[:, :], in0=pt[:, :], scalar=ty[:, bb, k : k + 1],
                        in1=hx[:, 0:256], op0=AL.mult, op1=AL.add,
                    )
                    for ch in range(2):
                        nc.tensor.transpose(
                            ps_b[:, k, ch, :],
                            pt[:, ch * 128 : (ch + 1) * 128],
                            ident_b[:, :],
                        )
                nc.scalar.copy(
                    out=rhs_buf[:, :, :, bb * SLOT : (bb + 1) * SLOT],
                    in_=ps_b[:, :, :, :],
                )

        # ---- main conv matmul ----
        out_sb = singles.tile([128, 2, NSLOT], F32)
        with tc.tile_pool(name="pm", bufs=4, space="PSUM") as pm:
            for ot in range(2):
                pms = [pm.tile([128, NSL], F32, tag="pmtile", name="pmtile") for _ in range(4)]
                for kc in range(18):
                    k, ch = divmod(kc, 2)
                    for ns in range(4):
                        nc.tensor.matmul(
                            pms[ns][:, :],
                            wm[:, k, ch, ot, :],
                            rhs_buf[:, k, ch, ns * NSL : (ns + 1) * NSL],
                            start=(kc == 0),
                            stop=(kc == 17),
                        )
                for ns in range(4):
                    nc.scalar.copy(
                        out=out_sb[:, ot, ns * NSL : (ns + 1) * NSL],
                        in_=pms[ns][:, :],
                    )
        nc.sync.dma_start(out=out_d[:, :, :], in_=out_sb[:, :, :])

    nc.compile()
    return nc


def prep_inputs(x, w_off, b_off, w):
    """Host-side slab/layout prep. Returns list of 8 per-core input dicts."""
    x = np.asarray(x, dtype=np.float32)
    w_off = np.asarray(w_off, dtype=np.float32)
    b_off = np.asarray(b_off, dtype=np.float32)
    w = np.asarray(w, dtype=np.float32)

    woff_arr = np.ascontiguousarray(
        w_off.reshape(18, 2, 128, KK).transpose(2, 1, 3, 0)
    ).astype(ml_dtypes.bfloat16)  # [128 cl, 2 ch, 9 k, 18 o]
    boff_arr = np.ascontiguousarray(b_off.reshape(18, 1))
    wm_arr = np.ascontiguousarray(
        w.reshape(2, 128, 2, 128, KK).transpose(3, 4, 2, 0, 1)
    ).astype(ml_dtypes.bfloat16)  # [128 cl, 9 k, 2 ch, 2 ot, 128 ol]

    in_maps = []
    for core in range(8):
        b, half = divmod(core, 2)
        r0 = half * NROWS
        xb = x[b]  # [256, 56, 56]

        xp58 = np.zeros((CIN, 58, 58), np.float32)
        xp58[:, 1:57, 1:57] = xb
        xcf = np.ascontiguousarray(
            xp58[:, r0 : r0 + 30, :].reshape(2, 128, 30 * 58).transpose(1, 0, 2)
        ).astype(ml_dtypes.bfloat16)

        xp = np.zeros((HQ + 1, WQ + 1, CIN), np.float32)
        ylo = max(0, r0 - MARG)
        yhi = min(H, r0 + HQ + 1 - MARG)
        xhwc = xb.transpose(1, 2, 0)
        xp[ylo - (r0 - MARG) : yhi - (r0 - MARG), MARG : MARG + W, :] = xhwc[ylo:yhi]
        quad = np.stack(
            [xp[:-1, :-1], xp[:-1, 1:], xp[1:, :-1], xp[1:, 1:]], axis=2
        )  # [72, 72, 4, 256]
        xq = np.ascontiguousarray(quad.reshape(NQ, 4 * CIN)).astype(ml_dtypes.bfloat16)

        in_maps.append(
            {
                "xcf": xcf,
                "xq": xq,
                "woff": woff_arr,
                "boff": boff_arr,
                "wm": wm_arr,
            }
        )
    return in_maps


def unshard_output(results):
    """results: list of 8 per-core out arrays [128, 2, NSLOT] -> [B,COUT,H,W]."""
    out = np.zeros((B, COUT, H, W), np.float32)
    for core in range(8):
        b, half = divmod(core, 2)
        r0 = half * NROWS
        oc = results[core]  # [128 ol, 2 ot, 1792]
        oc = oc.reshape(128, 2, NBLK, SLOT)[:, :, :, :BLK]
        oc = oc.transpose(1, 0, 2, 3).reshape(COUT, NROWS, W)
        out[b, :, r0 : r0 + NROWS, :] = oc
    return out


def kernel(**inputs):
    nc = _CACHE.get("nc")
    if nc is None:
        nc = build_nc()
        _CACHE["nc"] = nc
    in_maps = prep_inputs(
        inputs["x"], inputs["w_off"], inputs["b_off"], inputs["w"]
    )
    res = run_bass_kernel_spmd(nc, in_maps, core_ids=list(range(8)))
    return unshard_output([r["out"] for r in res.results])



# revision 10
# speedup vs baseline: 1.4867x; 1.4318x over previous
"""Deformable conv (nn_DeformConv_31267361915085) Trainium2 Bass kernel, v2.

Sharding: data-parallel over (batch, H-half): core n handles batch n//2,
output rows [28*(n%2), 28*(n%2)+28). Weights replicated. SPMD: one program;
per-core input slabs are pre-shifted on host so the program is core-agnostic.

v2 pipeline (per core, 13 blocks x 128 raster pixels):
  1. offset conv: 9 taps x 2 c-chunks of bf16 matmuls, PSUM-accumulated
     (PE pre-warmed with junk matmuls so it ramps to full clock).
  2. PE-transpose offsets to pixel-on-partition layout; coordinate math and
     bilinear corner weights (alpha) on DVE.
  3. per block: SWDGE dma_gather of 2x2 "quad" corner vectors (bf16, 2KB
     elements) from a zero-padded channels-last quad table in DRAM.
     Gathers round-robin over 4 SWDGE queues so Q7 descriptor generation
     runs on 4 core-pairs in parallel.
  4. bilinear lerp is fused into the (mandatory) patch transpose on the
     TENSOR engine: pt^T[c,p] = sum_q g_q^T @ diag(alpha_q).  diag(alpha)
     tiles are built on DVE as ident * alpha (one tensor_tensor per block).
  5. per block main conv: 36 bf16 matmuls, PSUM accumulate, DMA out.
"""

import sys

if "/opt/trn_rl_repo" not in sys.path:
    sys.path.insert(0, "/opt/trn_rl_repo")

import contextlib

import numpy as np
import ml_dtypes

import concourse.bass as bass
import concourse.tile as tile
from concourse import bacc, mybir
from concourse.bass_utils import run_bass_kernel_spmd
from concourse.masks import make_identity

F32 = mybir.dt.float32
BF16 = mybir.dt.bfloat16
I16 = mybir.dt.int16
I32 = mybir.dt.int32
AL = mybir.AluOpType

# problem dims
B, CIN, H, W = 4, 256, 56, 56
COUT = 256
KK = 9
MARG = 8                # gather pad margin (covers |offset| <= ~6)
HQ = WQ = H + 2 * MARG  # 72: quad-table grid
NQ = HQ * WQ            # 5184 quad rows
NROWS = 28              # output rows per core
NPIX = NROWS * W        # 1568
BLK = 128               # pixels per block (raster order)
NBLK = 13               # ceil(1568/128) = 12.25 -> 13 (last block 96 pad)
NSLOT = NBLK * BLK      # 1664
NIDX = KK * BLK         # 1152 gather indices per block

_CACHE = {}


def _ap(base, offset_elems, dims):
    """AP with explicit free dims on top of a tile's base AP."""
    return bass.AP(
        tensor=base.tensor, offset=base.offset + offset_elems, ap=[base.ap[0]] + dims
    )


def build_nc():
    nc = bacc.Bacc(None, target_bir_lowering=False, num_swdge_queues=4)

    xcf_d = nc.dram_tensor("xcf", [128, 2, 30 * 58], BF16, kind="ExternalInput")
    xq_d = nc.dram_tensor("xq", [NQ, 1024], BF16, kind="ExternalInput")
    woff_d = nc.dram_tensor("woff", [128, 2, KK, 18], BF16, kind="ExternalInput")
    boff_d = nc.dram_tensor("boff", [18, 1], F32, kind="ExternalInput")
    wm_d = nc.dram_tensor("wm", [128, KK, 2, 2, 128], BF16, kind="ExternalInput")
    out_d = nc.dram_tensor("out", [128, 2, NSLOT], F32, kind="ExternalOutput")

    with tile.TileContext(nc) as tc, contextlib.ExitStack() as ctx:
        singles = ctx.enter_context(tc.tile_pool(name="singles", bufs=1))
        coords = ctx.enter_context(tc.tile_pool(name="coords", bufs=1))

        # ---- load constants / weights / activations ----
        xcf = singles.tile([128, 2, 30 * 58], BF16)
        nc.sync.dma_start(out=xcf[:, :, :], in_=xcf_d[:, :, :])
        woff = singles.tile([128, 2, KK, 18], BF16)
        nc.sync.dma_start(out=woff[:, :, :, :], in_=woff_d[:, :, :, :])
        boff = singles.tile([18, 1], F32)
        nc.sync.dma_start(out=boff[:, :], in_=boff_d[:, :])
        wm = singles.tile([128, KK, 2, 2, 128], BF16)
        nc.sync.dma_start(out=wm[:, :, :, :, :], in_=wm_d[:, :, :, :, :])

        ident_f = singles.tile([128, 128], F32)
        make_identity(nc, ident_f[:, :])
        ident_b = singles.tile([128, 128], BF16)
        nc.vector.tensor_copy(out=ident_b[:, :], in_=ident_f[:, :])

        # ---- PE warmup: ramp the clock while input DMAs land ----
        with tc.tile_pool(name="pwarm", bufs=1, space="PSUM") as pw:
            wps = pw.tile([128, 128], F32)
            for _ in range(8):
                nc.tensor.matmul(
                    wps[:, :], ident_f[:, :], ident_f[:, :], start=True, stop=True
                )

        # ---- offset conv: off_sb [18, NSLOT] f32, raster pixel cols ----
        off_sb = coords.tile([18, NSLOT], F32)
        nc.vector.memset(off_sb[:, NPIX:NSLOT], 0.0)
        with tc.tile_pool(name="po", bufs=2, space="PSUM") as po:
            for ns in range(4):
                ps_o = po.tile([18, 392], F32)
                for kc in range(18):
                    k, ch = divmod(kc, 2)
                    ky, kx = divmod(k, 3)
                    rhs = _ap(
                        xcf[:, :, :],
                        ch * 1740 + (ns * 7 + ky) * 58 + kx,
                        [[58, 7], [1, 56]],
                    )
                    nc.tensor.matmul(
                        ps_o[:, :],
                        woff[:, ch, k, :],
                        rhs,
                        start=(kc == 0),
                        stop=(kc == 17),
                    )
                nc.vector.tensor_scalar(
                    out=off_sb[:, ns * 392 : (ns + 1) * 392],
                    in0=ps_o[:, :],
                    scalar1=boff[:, 0:1],
                    scalar2=None,
                    op0=AL.add,
                )

        # ---- transpose offsets to pixel-on-partition [128, NBLK, 18] ----
        offT = coords.tile([128, NBLK, 18], F32)
        with tc.tile_pool(name="pot", bufs=1, space="PSUM") as pot:
            ps_t = pot.tile([128, NBLK, 18], F32)
            for bb in range(NBLK):
                nc.tensor.transpose(
                    ps_t[:, bb, :],
                    off_sb[:18, bb * BLK : (bb + 1) * BLK],
                    ident_f[:18, :18],
                )
            nc.vector.tensor_copy(out=offT[:, :, :], in_=ps_t[:, :, :])

        # ---- coordinate + weight math (fp32 [128, NBLK, 9] planes) ----
        _fc = [0]

        def floor_fix(dst_f, src, shape):
            """dst_f = floor(src) for src >= 0 (i32 round-to-nearest + fixup)."""
            _fc[0] += 1
            sl = (slice(None),) * len(shape)
            ci = coords.tile(shape, I32, name=f"ci{_fc[0]}")
            nc.vector.tensor_copy(out=ci[sl], in_=src[sl])
            nc.vector.tensor_copy(out=dst_f[sl], in_=ci[sl])
            gt = coords.tile(shape, F32, name=f"gt{_fc[0]}")
            nc.vector.tensor_tensor(
                out=gt[sl], in0=dst_f[sl], in1=src[sl], op=AL.is_gt
            )
            nc.vector.tensor_tensor(
                out=dst_f[sl], in0=dst_f[sl], in1=gt[sl], op=AL.subtract
            )

        # p = 128*bb + s; r = p//56; j = p%56
        p_i = coords.tile([128, NBLK], I32)
        nc.gpsimd.iota(p_i[:, :], pattern=[[BLK, NBLK]], base=0, channel_multiplier=1)
        p_f = coords.tile([128, NBLK], F32)
        nc.vector.tensor_copy(out=p_f[:, :], in_=p_i[:, :])
        t56 = coords.tile([128, NBLK], F32)
        nc.vector.tensor_scalar(
            out=t56[:, :], in0=p_f[:, :], scalar1=0.5, scalar2=1.0 / 56.0,
            op0=AL.add, op1=AL.mult,
        )
        r_f = coords.tile([128, NBLK], F32)
        floor_fix(r_f, t56, [128, NBLK])
        jx = coords.tile([128, NBLK], F32)
        nc.vector.scalar_tensor_tensor(
            out=jx[:, :], in0=r_f[:, :], scalar=-56.0, in1=p_f[:, :],
            op0=AL.mult, op1=AL.add,
        )

        kyM_i = coords.tile([128, KK], I32)
        nc.gpsimd.iota(
            kyM_i[:, :], pattern=[[1, 3], [0, 3]], base=MARG - 1, channel_multiplier=0
        )
        kyM = coords.tile([128, KK], F32)
        nc.vector.tensor_copy(out=kyM[:, :], in_=kyM_i[:, :])
        kxM_i = coords.tile([128, KK], I32)
        nc.gpsimd.iota(
            kxM_i[:, :], pattern=[[0, 3], [1, 3]], base=MARG - 1, channel_multiplier=0
        )
        kxM = coords.tile([128, KK], F32)
        nc.vector.tensor_copy(out=kxM[:, :], in_=kxM_i[:, :])

        dy = _ap(offT[:], 0, [[18, NBLK], [2, KK]])
        dx = _ap(offT[:], 1, [[18, NBLK], [2, KK]])
        r_b = _ap(r_f[:], 0, [[1, NBLK], [0, KK]])
        jx_b = _ap(jx[:], 0, [[1, NBLK], [0, KK]])
        kyM_b = _ap(kyM[:], 0, [[0, NBLK], [1, KK]])
        kxM_b = _ap(kxM[:], 0, [[0, NBLK], [1, KK]])

        P3 = [128, NBLK, KK]
        pym = coords.tile(P3, F32)
        pxm = coords.tile(P3, F32)
        nc.vector.tensor_tensor(out=pym[:, :, :], in0=dy, in1=r_b, op=AL.add)
        nc.vector.tensor_tensor(out=pym[:, :, :], in0=pym[:, :, :], in1=kyM_b, op=AL.add)
        nc.vector.tensor_tensor(out=pxm[:, :, :], in0=dx, in1=jx_b, op=AL.add)
        nc.vector.tensor_tensor(out=pxm[:, :, :], in0=pxm[:, :, :], in1=kxM_b, op=AL.add)

        y0 = coords.tile(P3, F32)
        x0 = coords.tile(P3, F32)
        floor_fix(y0, pym, P3)
        floor_fix(x0, pxm, P3)
        ty = coords.tile(P3, F32)
        tx = coords.tile(P3, F32)
        nc.vector.tensor_tensor(
            out=ty[:, :, :], in0=pym[:, :, :], in1=y0[:, :, :], op=AL.subtract
        )
        nc.vector.tensor_tensor(
            out=tx[:, :, :], in0=pxm[:, :, :], in1=x0[:, :, :], op=AL.subtract
        )

        # bilinear corner weights: q order (a,b,c,d) matches quad packing
        u = coords.tile(P3, F32)  # 1 - tx
        v = coords.tile(P3, F32)  # 1 - ty
        nc.vector.tensor_scalar(
            out=u[:, :, :], in0=tx[:, :, :], scalar1=-1.0, scalar2=1.0,
            op0=AL.mult, op1=AL.add,
        )
        nc.vector.tensor_scalar(
            out=v[:, :, :], in0=ty[:, :, :], scalar1=-1.0, scalar2=1.0,
            op0=AL.mult, op1=AL.add,
        )
        alphas = coords.tile([128, NBLK, KK, 4], F32)
        for q, (fy, fx_) in enumerate(((v, u), (v, tx), (ty, u), (ty, tx))):
            nc.vector.tensor_tensor(
                out=_ap(alphas[:], q, [[KK * 4, NBLK], [4, KK]]),
                in0=fy[:, :, :],
                in1=fx_[:, :, :],
                op=AL.mult,
            )

        # quad-table row index
        idxf = coords.tile(P3, F32)
        nc.vector.scalar_tensor_tensor(
            out=idxf[:, :, :], in0=y0[:, :, :], scalar=float(WQ), in1=x0[:, :, :],
            op0=AL.mult, op1=AL.add,
        )

        # ---- fold indices into SWDGE wrapped layout ----
        # idxw[16m+q, bb, k*8+t] = idx[s=16t+q, bb, k]
        idxT16 = coords.tile([117, 16, 8], I16)
        with tc.tile_pool(name="pidx", bufs=1, space="PSUM") as pidx:
            ps_i = pidx.tile([117, 128], F32)
            nc.tensor.transpose(
                ps_i[:, :], _ap(idxf[:, :, :], 0, [[1, 117]]), ident_f[:, :]
            )
            # permute columns to wrap order: dst col q*8+t <- pixel 16t+q
            nc.vector.tensor_copy(
                out=idxT16[:, :, :], in_=_ap(ps_i[:, :], 0, [[1, 16], [16, 8]])
            )

        idxw = coords.tile([128, NBLK, 72], I16)
        ppw = idxw[:, :, :].ap[0][0]
        ppt = idxT16[:, :, :].ap[0][0]
        for q in range(16):
            src = bass.AP(
                tensor=idxT16.tensor,
                offset=idxT16.offset + q * 8,
                ap=[[ppt, 117], [1, 8]],
            )
            dst = bass.AP(
                tensor=idxw.tensor,
                offset=idxw.offset + q * ppw,
                ap=[[ppw, 1], [72, NBLK], [8, KK], [1, 8]],
            )
            nc.sync.dma_start(out=dst, in_=src)
        rep = NBLK * 72
        for m in (16, 32, 64):
            src = bass.AP(tensor=idxw.tensor, offset=idxw.offset, ap=[[ppw, m], [1, rep]])
            dst = bass.AP(
                tensor=idxw.tensor,
                offset=idxw.offset + m * ppw,
                ap=[[ppw, m], [1, rep]],
            )
            nc.sync.dma_start(out=dst, in_=src)

        # ---- per-block: gather -> diag -> scaled transposes -> main conv ----
        with (
            tc.tile_pool(name="gp", bufs=4) as gp,
            tc.tile_pool(name="dp", bufs=2) as dp,
            tc.tile_pool(name="rp", bufs=2) as rp,
            tc.tile_pool(name="osb", bufs=2) as osb,
            tc.tile_pool(name="ptp", bufs=2, space="PSUM") as ptp,
            tc.tile_pool(name="oup", bufs=2, space="PSUM") as oup,
        ):
            for bb in range(NBLK):
                g = gp.tile([128, KK, 1024], BF16)
                nc.gpsimd.dma_gather(
                    out_ap=g[:, :, :],
                    in_ap=xq_d[:, :],
                    idxs_ap=idxw[:, bb, :],
                    num_idxs=NIDX,
                    num_idxs_reg=NIDX,
                    elem_size=1024,
                    single_packet=False,
                    queue_num=bb % 4,
                )
                diag = dp.tile([128, KK, 4, 128], BF16)
                nc.vector.tensor_tensor(
                    out=diag[:, :, :, :],
                    in0=_ap(ident_b[:, :], 0, [[0, KK], [0, 4], [1, 128]]),
                    in1=_ap(alphas[:, :, :, :], bb * KK * 4, [[4, KK], [1, 4], [0, 128]]),
                    op=AL.mult,
                )
                rhs_t = rp.tile([128, KK, 2, 128], BF16)
                for grp in range(3):
                    pt = ptp.tile(
                        [128, 3, 2, 128], F32,
                        tag="pt", name="pt",
                    )
                    for kk in range(3):
                        k = grp * 3 + kk
                        for ch in range(2):
                            for q in range(4):
                                nc.tensor.matmul(
                                    pt[:, kk, ch, :],
                                    _ap(g[:, :, :], k * 1024 + q * 256 + ch * 128, [[1, 128]]),
                                    diag[:, k, q, :],
                                    start=(q == 0),
                                    stop=(q == 3),
                                )
                    nc.scalar.copy(
                        out=rhs_t[:, grp * 3 : (grp + 1) * 3, :, :],
                        in_=pt[:, :, :, :],
                    )
                # each oh group gets its own 2KB PSUM bank: matmul start=True
                # zeroes the whole bank, so interleaved groups must not share
                outp = oup.tile([128, 2, 512], F32)
                for kc in range(18):
                    k, ch = divmod(kc, 2)
                    for oh in range(2):
                        nc.tensor.matmul(
                            outp[:, oh, 0:128],
                            wm[:, k, ch, oh, :],
                            rhs_t[:, k, ch, :],
                            start=(kc == 0),
                            stop=(kc == 17),
                        )
                o_t = osb.tile([128, 2, 128], F32)
                nc.vector.tensor_copy(out=o_t[:, :, :], in_=outp[:, :, 0:128])
                nc.sync.dma_start(
                    out=out_d[:, :, bb * BLK : (bb + 1) * BLK], in_=o_t[:, :, :]
                )

    nc.compile()
    return nc


def prep_inputs(x, w_off, b_off, w):
    """Host-side slab/layout prep. Returns list of 8 per-core input dicts."""
    x = np.asarray(x, dtype=np.float32)
    w_off = np.asarray(w_off, dtype=np.float32)
    b_off = np.asarray(b_off, dtype=np.float32)
    w = np.asarray(w, dtype=np.float32)

    woff_arr = np.ascontiguousarray(
        w_off.reshape(18, 2, 128, KK).transpose(2, 1, 3, 0)
    ).astype(ml_dtypes.bfloat16)  # [128 cl, 2 ch, 9 k, 18 o]
    boff_arr = np.ascontiguousarray(b_off.reshape(18, 1))
    wm_arr = np.ascontiguousarray(
        w.reshape(2, 128, 2, 128, KK).transpose(3, 4, 2, 0, 1)
    ).astype(ml_dtypes.bfloat16)  # [128 cl, 9 k, 2 ch, 2 ot, 128 ol]

    in_maps = []
    for core in range(8):
        b, half = divmod(core, 2)
        r0 = half * NROWS
        xb = x[b]  # [256, 56, 56]

        xp58 = np.zeros((CIN, 58, 58), np.float32)
        xp58[:, 1:57, 1:57] = xb
        xcf = np.ascontiguousarray(
            xp58[:, r0 : r0 + 30, :].reshape(2, 128, 30 * 58).transpose(1, 0, 2)
        ).astype(ml_dtypes.bfloat16)

        xp = np.zeros((HQ + 1, WQ + 1, CIN), np.float32)
        ylo = max(0, r0 - MARG)
        yhi = min(H, r0 + HQ + 1 - MARG)
        xhwc = xb.transpose(1, 2, 0)
        xp[ylo - (r0 - MARG) : yhi - (r0 - MARG), MARG : MARG + W, :] = xhwc[ylo:yhi]
        quad = np.stack(
            [xp[:-1, :-1], xp[:-1, 1:], xp[1:, :-1], xp[1:, 1:]], axis=2
        )  # [72, 72, 4, 256]
        xq = np.ascontiguousarray(quad.reshape(NQ, 4 * CIN)).astype(ml_dtypes.bfloat16)

        in_maps.append(
            {
                "xcf": xcf,
                "xq": xq,
                "woff": woff_arr,
                "boff": boff_arr,
                "wm": wm_arr,
            }
        )
    return in_maps


def unshard_output(results):
    """results: list of 8 per-core out arrays [128, 2, NSLOT] -> [B,COUT,H,W]."""
    out = np.zeros((B, COUT, H, W), np.float32)
    for core in range(8):
        b, half = divmod(core, 2)
        r0 = half * NROWS
        oc = results[core]  # [128 ol, 2 oh, NSLOT]
        oc = oc.transpose(1, 0, 2).reshape(COUT, NSLOT)[:, :NPIX]
        out[b, :, r0 : r0 + NROWS, :] = oc.reshape(COUT, NROWS, W)
    return out


def kernel(**inputs):
    nc = _CACHE.get("nc")
    if nc is None:
        nc = build_nc()
        _CACHE["nc"] = nc
    in_maps = prep_inputs(
        inputs["x"], inputs["w_off"], inputs["b_off"], inputs["w"]
    )
    res = run_bass_kernel_spmd(nc, in_maps, core_ids=list(range(8)))
    return unshard_output([r["out"] for r in res.results])


# revision 20
# speedup vs baseline: 2.1386x; 1.4384x over previous
"""Deformable conv (nn_DeformConv_31267361915085) Trainium2 Bass kernel, v2.

Sharding: data-parallel over (batch, H-half): core n handles batch n//2,
output rows [28*(n%2), 28*(n%2)+28). Weights replicated. SPMD: one program;
per-core input slabs are pre-shifted on host so the program is core-agnostic.

v2 pipeline (per core, 13 blocks x 128 raster pixels):
  1. offset conv: 9 taps x 2 c-chunks of bf16 matmuls, PSUM-accumulated
     (PE pre-warmed with junk matmuls so it ramps to full clock).
  2. PE-transpose offsets to pixel-on-partition layout; coordinate math and
     bilinear corner weights (alpha) on DVE.
  3. per block: SWDGE dma_gather of 2x2 "quad" corner vectors (bf16, 2KB
     elements) from a zero-padded channels-last quad table in DRAM.
     Gathers round-robin over 4 SWDGE queues so Q7 descriptor generation
     runs on 4 core-pairs in parallel.
  4. bilinear lerp is fused into the (mandatory) patch transpose on the
     TENSOR engine: pt^T[c,p] = sum_q g_q^T @ diag(alpha_q).  diag(alpha)
     tiles are built on DVE as ident * alpha (one tensor_tensor per block).
  5. per block main conv: 36 bf16 matmuls, PSUM accumulate, DMA out.
"""

import sys

if "/opt/trn_rl_repo" not in sys.path:
    sys.path.insert(0, "/opt/trn_rl_repo")

import contextlib

import numpy as np
import ml_dtypes

import concourse.bass as bass
import concourse.tile as tile
from concourse import bacc, mybir
from concourse.bass_utils import run_bass_kernel_spmd
from concourse.masks import make_identity

F32 = mybir.dt.float32
BF16 = mybir.dt.bfloat16
F8E3 = mybir.dt.float8e3
I16 = mybir.dt.int16
I32 = mybir.dt.int32
AL = mybir.AluOpType

# problem dims
B, CIN, H, W = 4, 256, 56, 56
COUT = 256
KK = 9
MARG = 8                # gather pad margin (covers |offset| <= ~6)
HQ = WQ = H + 2 * MARG  # 72: quad-table grid
NQ = HQ * WQ            # 5184 quad rows
NROWS = 28              # output rows per core
NPIX = NROWS * W        # 1568
BLK = 128               # pixels per block (raster order)
NBLK = 13               # ceil(1568/128) = 12.25 -> 13 (last block 96 pad)
NSLOT = NBLK * BLK      # 1664
NIDX = KK * BLK         # 1152 gather indices per block

_CACHE = {}


def _ap(base, offset_elems, dims):
    """AP with explicit free dims on top of a tile's base AP."""
    return bass.AP(
        tensor=base.tensor, offset=base.offset + offset_elems, ap=[base.ap[0]] + dims
    )


def build_nc():
    nc = bacc.Bacc(None, target_bir_lowering=False, num_swdge_queues=4)

    xcf_d = nc.dram_tensor("xcf", [128, 2, 30 * 58], BF16, kind="ExternalInput")
    xq_d = nc.dram_tensor("xq", [NQ, 1024], F8E3, kind="ExternalInput")
    woff_d = nc.dram_tensor("woff", [128, 2, KK, 18], BF16, kind="ExternalInput")
    boff_d = nc.dram_tensor("boff", [18, 1], F32, kind="ExternalInput")
    wm_d = nc.dram_tensor("wm", [128, KK, 2, 2, 128], BF16, kind="ExternalInput")
    out_d = nc.dram_tensor("out", [128, 2, NSLOT], F32, kind="ExternalOutput")

    with tile.TileContext(nc) as tc, contextlib.ExitStack() as ctx:
        singles = ctx.enter_context(tc.tile_pool(name="singles", bufs=1))
        coords = ctx.enter_context(tc.tile_pool(name="coords", bufs=1))

        # ---- load constants / weights / activations ----
        xcf = singles.tile([128, 2, 30 * 58], BF16)
        nc.sync.dma_start(out=xcf[:, :, :], in_=xcf_d[:, :, :])
        woff = singles.tile([128, 2, KK, 18], BF16)
        nc.sync.dma_start(out=woff[:, :, :, :], in_=woff_d[:, :, :, :])
        boff = singles.tile([18, 1], F32)
        nc.sync.dma_start(out=boff[:, :], in_=boff_d[:, :])
        wm = singles.tile([128, KK, 2, 2, 128], BF16)
        nc.sync.dma_start(out=wm[:, :, :, :, :], in_=wm_d[:, :, :, :, :])

        ident_f = singles.tile([128, 128], F32)
        make_identity(nc, ident_f[:, :])
        ident_b = singles.tile([128, 128], BF16)
        nc.vector.tensor_copy(out=ident_b[:, :], in_=ident_f[:, :])

        # ---- PE warmup: ramp the clock while input DMAs land ----
        with tc.tile_pool(name="pwarm", bufs=1, space="PSUM") as pw:
            wps = pw.tile([128, 128], F32)
            for _ in range(8):
                nc.tensor.matmul(
                    wps[:, :], ident_f[:, :], ident_f[:, :], start=True, stop=True
                )

        # ---- offset conv: off_sb [18, NSLOT] f32, raster pixel cols ----
        off_sb = coords.tile([18, NSLOT], F32)
        nc.vector.memset(off_sb[:, NPIX:NSLOT], 0.0)
        with tc.tile_pool(name="po", bufs=2, space="PSUM") as po:
            for ns in range(4):
                ps_o = po.tile([18, 392], F32)
                for kc in range(18):
                    k, ch = divmod(kc, 2)
                    ky, kx = divmod(k, 3)
                    rhs = _ap(
                        xcf[:, :, :],
                        ch * 1740 + (ns * 7 + ky) * 58 + kx,
                        [[58, 7], [1, 56]],
                    )
                    nc.tensor.matmul(
                        ps_o[:, :],
                        woff[:, ch, k, :],
                        rhs,
                        start=(kc == 0),
                        stop=(kc == 17),
                    )
                nc.vector.tensor_scalar(
                    out=off_sb[:, ns * 392 : (ns + 1) * 392],
                    in0=ps_o[:, :],
                    scalar1=boff[:, 0:1],
                    scalar2=None,
                    op0=AL.add,
                )

        # ---- transpose offsets to pixel-on-partition [128, NBLK, 18] ----
        offT = coords.tile([128, NBLK, 18], F32)
        with tc.tile_pool(name="pot", bufs=1, space="PSUM") as pot:
            ps_t = pot.tile([128, NBLK, 18], F32)
            for bb in range(NBLK):
                nc.tensor.transpose(
                    ps_t[:, bb, :],
                    off_sb[:18, bb * BLK : (bb + 1) * BLK],
                    ident_f[:18, :18],
                )
            nc.vector.tensor_copy(out=offT[:, :, :], in_=ps_t[:, :, :])

        # ---- coordinate + weight math (fp32 [128, NBLK, 9] planes) ----
        _fc = [0]

        def floor_fix(dst_f, src, shape):
            """dst_f = floor(src) for src >= 0 (i32 round-to-nearest + fixup)."""
            _fc[0] += 1
            sl = (slice(None),) * len(shape)
            ci = coords.tile(shape, I32, name=f"ci{_fc[0]}")
            nc.vector.tensor_copy(out=ci[sl], in_=src[sl])
            nc.vector.tensor_copy(out=dst_f[sl], in_=ci[sl])
            gt = coords.tile(shape, F32, name=f"gt{_fc[0]}")
            nc.vector.tensor_tensor(
                out=gt[sl], in0=dst_f[sl], in1=src[sl], op=AL.is_gt
            )
            nc.vector.tensor_tensor(
                out=dst_f[sl], in0=dst_f[sl], in1=gt[sl], op=AL.subtract
            )

        # p = 128*bb + s; r = p//56; j = p%56
        p_i = coords.tile([128, NBLK], I32)
        nc.gpsimd.iota(p_i[:, :], pattern=[[BLK, NBLK]], base=0, channel_multiplier=1)
        p_f = coords.tile([128, NBLK], F32)
        nc.vector.tensor_copy(out=p_f[:, :], in_=p_i[:, :])
        t56 = coords.tile([128, NBLK], F32)
        nc.vector.tensor_scalar(
            out=t56[:, :], in0=p_f[:, :], scalar1=0.5, scalar2=1.0 / 56.0,
            op0=AL.add, op1=AL.mult,
        )
        r_f = coords.tile([128, NBLK], F32)
        floor_fix(r_f, t56, [128, NBLK])
        jx = coords.tile([128, NBLK], F32)
        nc.vector.scalar_tensor_tensor(
            out=jx[:, :], in0=r_f[:, :], scalar=-56.0, in1=p_f[:, :],
            op0=AL.mult, op1=AL.add,
        )

        kyM_i = coords.tile([128, KK], I32)
        nc.gpsimd.iota(
            kyM_i[:, :], pattern=[[1, 3], [0, 3]], base=MARG - 1, channel_multiplier=0
        )
        kyM = coords.tile([128, KK], F32)
        nc.vector.tensor_copy(out=kyM[:, :], in_=kyM_i[:, :])
        kxM_i = coords.tile([128, KK], I32)
        nc.gpsimd.iota(
            kxM_i[:, :], pattern=[[0, 3], [1, 3]], base=MARG - 1, channel_multiplier=0
        )
        kxM = coords.tile([128, KK], F32)
        nc.vector.tensor_copy(out=kxM[:, :], in_=kxM_i[:, :])

        dy = _ap(offT[:], 0, [[18, NBLK], [2, KK]])
        dx = _ap(offT[:], 1, [[18, NBLK], [2, KK]])
        r_b = _ap(r_f[:], 0, [[1, NBLK], [0, KK]])
        jx_b = _ap(jx[:], 0, [[1, NBLK], [0, KK]])
        kyM_b = _ap(kyM[:], 0, [[0, NBLK], [1, KK]])
        kxM_b = _ap(kxM[:], 0, [[0, NBLK], [1, KK]])

        P3 = [128, NBLK, KK]
        pym = coords.tile(P3, F32)
        pxm = coords.tile(P3, F32)
        # first add walks (k outer, bb inner) so the broadcast operand has a
        # stride-1 innermost dim (a 0-stride innermost dim is ~30x slower)
        dy_kb = _ap(offT[:], 0, [[2, KK], [18, NBLK]])
        dx_kb = _ap(offT[:], 1, [[2, KK], [18, NBLK]])
        r_kb = _ap(r_f[:], 0, [[0, KK], [1, NBLK]])
        jx_kb = _ap(jx[:], 0, [[0, KK], [1, NBLK]])
        pym_kb = _ap(pym[:], 0, [[1, KK], [KK, NBLK]])
        pxm_kb = _ap(pxm[:], 0, [[1, KK], [KK, NBLK]])
        nc.vector.tensor_tensor(out=pym_kb, in0=dy_kb, in1=r_kb, op=AL.add)
        nc.vector.tensor_tensor(out=pym[:, :, :], in0=pym[:, :, :], in1=kyM_b, op=AL.add)
        nc.vector.tensor_tensor(out=pxm_kb, in0=dx_kb, in1=jx_kb, op=AL.add)
        nc.vector.tensor_tensor(out=pxm[:, :, :], in0=pxm[:, :, :], in1=kxM_b, op=AL.add)

        y0 = coords.tile(P3, F32)
        x0 = coords.tile(P3, F32)
        floor_fix(y0, pym, P3)
        floor_fix(x0, pxm, P3)
        ty = coords.tile(P3, F32)
        tx = coords.tile(P3, F32)
        nc.vector.tensor_tensor(
            out=ty[:, :, :], in0=pym[:, :, :], in1=y0[:, :, :], op=AL.subtract
        )
        nc.vector.tensor_tensor(
            out=tx[:, :, :], in0=pxm[:, :, :], in1=x0[:, :, :], op=AL.subtract
        )

        # bilinear corner weights: q order (a,b,c,d) matches quad packing
        u = coords.tile(P3, F32)  # 1 - tx
        v = coords.tile(P3, F32)  # 1 - ty
        nc.vector.tensor_scalar(
            out=u[:, :, :], in0=tx[:, :, :], scalar1=-1.0, scalar2=1.0,
            op0=AL.mult, op1=AL.add,
        )
        nc.vector.tensor_scalar(
            out=v[:, :, :], in0=ty[:, :, :], scalar1=-1.0, scalar2=1.0,
            op0=AL.mult, op1=AL.add,
        )
        # [128, 4 q, NBLK, KK]: q-major so each product writes contiguously
        alphas = coords.tile([128, 4, NBLK, KK], F32)
        for q, (fy, fx_) in enumerate(((v, u), (v, tx), (ty, u), (ty, tx))):
            nc.vector.tensor_tensor(
                out=alphas[:, q, :, :],
                in0=fy[:, :, :],
                in1=fx_[:, :, :],
                op=AL.mult,
            )

        # quad-table row index
        idxf = coords.tile(P3, F32)
        nc.vector.scalar_tensor_tensor(
            out=idxf[:, :, :], in0=y0[:, :, :], scalar=float(WQ), in1=x0[:, :, :],
            op0=AL.mult, op1=AL.add,
        )

        # ---- fold indices into SWDGE wrapped layout ----
        # idxw[16m+q, bb, k*8+t] = idx[s=16t+q, bb, k]
        idxT16 = coords.tile([117, 16, 8], I16)
        with tc.tile_pool(name="pidx", bufs=1, space="PSUM") as pidx:
            ps_i = pidx.tile([117, 128], F32)
            nc.tensor.transpose(
                ps_i[:, :], _ap(idxf[:, :, :], 0, [[1, 117]]), ident_f[:, :]
            )
            # permute columns to wrap order: dst col q*8+t <- pixel 16t+q
            nc.vector.tensor_copy(
                out=idxT16[:, :, :], in_=_ap(ps_i[:, :], 0, [[1, 16], [16, 8]])
            )

        idxw = coords.tile([128, NBLK, 72], I16)
        ppw = idxw[:, :, :].ap[0][0]
        ppt = idxT16[:, :, :].ap[0][0]
        # spread the 16 fold DMAs over 4 engine queues (HWDGE issue cost is
        # ~0.8us per dma_start; serialized on one queue this was 14us)
        dma_engines = [nc.sync, nc.scalar]
        for q in range(16):
            src = bass.AP(
                tensor=idxT16.tensor,
                offset=idxT16.offset + q * 8,
                ap=[[ppt, 117], [1, 8]],
            )
            dst = bass.AP(
                tensor=idxw.tensor,
                offset=idxw.offset + q * ppw,
                ap=[[ppw, 1], [72, NBLK], [8, KK], [1, 8]],
            )
            dma_engines[q % 2].dma_start(out=dst, in_=src)
        rep = NBLK * 72
        # replicate group 0 -> groups 1..7 (independent copies, spread queues)
        for m in range(1, 8):
            src = bass.AP(tensor=idxw.tensor, offset=idxw.offset, ap=[[ppw, 16], [1, rep]])
            dst = bass.AP(
                tensor=idxw.tensor,
                offset=idxw.offset + 16 * m * ppw,
                ap=[[ppw, 16], [1, rep]],
            )
            dma_engines[m % 2].dma_start(out=dst, in_=src)

        # ---- per-block: gather -> diag -> scaled transposes -> main conv ----
        # main matmul batches PAIRS of blocks (256-col streams amortize the
        # wm weight loads); NBLK=13 so the last "pair" is a single block.
        with (
            tc.tile_pool(name="gp", bufs=6) as gp,
            tc.tile_pool(name="dp", bufs=2) as dp,
            tc.tile_pool(name="rp", bufs=2) as rp,
            tc.tile_pool(name="osb", bufs=2) as osb,
            tc.tile_pool(name="ptp", bufs=2, space="PSUM") as ptp,
            tc.tile_pool(name="oup", bufs=2, space="PSUM") as oup,
        ):
            rhs_t = None
            for bb in range(NBLK):
                half = bb % 2
                g = gp.tile([128, KK, 1024], F8E3)
                nc.gpsimd.dma_gather(
                    out_ap=g[:, :, :],
                    in_ap=xq_d[:, :],
                    idxs_ap=idxw[:, bb, :],
                    num_idxs=NIDX,
                    num_idxs_reg=NIDX,
                    elem_size=1024,
                    single_packet=False,
                    queue_num=bb % 4,
                )
                diag = dp.tile([128, KK, 4, 128], BF16)
                nc.vector.tensor_tensor(
                    out=diag[:, :, :, :],
                    in0=_ap(ident_b[:, :], 0, [[0, KK], [0, 4], [1, 128]]),
                    in1=_ap(
                        alphas[:, :, :, :], bb * KK,
                        [[1, KK], [NBLK * KK, 4], [0, 128]],
                    ),
                    op=AL.mult,
                )
                if half == 0:
                    rhs_t = rp.tile([128, KK, 2, 2, 128], BF16, tag="rhs", name="rhs")
                for grp in range(3):
                    pt = ptp.tile([128, 3, 2, 128], F32, tag="pt", name="pt")
                    for kk in range(3):
                        k = grp * 3 + kk
                        for ch in range(2):
                            for q in range(4):
                                nc.tensor.matmul(
                                    pt[:, kk, ch, :],
                                    _ap(g[:, :, :], k * 1024 + q * 256 + ch * 128, [[1, 128]]),
                                    diag[:, k, q, :],
                                    start=(q == 0),
                                    stop=(q == 3),
                                )
                    nc.scalar.copy(
                        out=rhs_t[:, grp * 3 : (grp + 1) * 3, :, half, :],
                        in_=pt[:, :, :, :],
                    )
                if half == 1 or bb == NBLK - 1:
                    ncols = (half + 1) * 128
                    b0 = bb - half
                    # each oh group gets its own 2KB PSUM bank (start=True
                    # zeroes the whole bank; groups must not share one)
                    outp = oup.tile([128, 2, 512], F32, tag="outp", name="outp")
                    for kc in range(18):
                        k, ch = divmod(kc, 2)
                        for oh in range(2):
                            nc.tensor.matmul(
                                outp[:, oh, 0:ncols],
                                wm[:, k, ch, oh, :],
                                _ap(rhs_t[:, :, :, :, :], (k * 2 + ch) * 256, [[1, ncols]]),
                                start=(kc == 0),
                                stop=(kc == 17),
                            )
                    o_t = osb.tile([128, 2, 256], F32, tag="ot", name="ot")
                    nc.scalar.copy(
                        out=o_t[:, :, 0:ncols], in_=outp[:, :, 0:ncols]
                    )
                    nc.sync.dma_start(
                        out=out_d[:, :, b0 * BLK : b0 * BLK + ncols],
                        in_=_ap(o_t[:, :, :], 0, [[256, 2], [1, ncols]]),
                    )

    nc.compile()
    return nc


def prep_inputs(x, w_off, b_off, w):
    """Host-side slab/layout prep. Returns list of 8 per-core input dicts."""
    x = np.asarray(x, dtype=np.float32)
    w_off = np.asarray(w_off, dtype=np.float32)
    b_off = np.asarray(b_off, dtype=np.float32)
    w = np.asarray(w, dtype=np.float32)

    woff_arr = np.ascontiguousarray(
        w_off.reshape(18, 2, 128, KK).transpose(2, 1, 3, 0)
    ).astype(ml_dtypes.bfloat16)  # [128 cl, 2 ch, 9 k, 18 o]
    boff_arr = np.ascontiguousarray(b_off.reshape(18, 1))
    wm_arr = np.ascontiguousarray(
        w.reshape(2, 128, 2, 128, KK).transpose(3, 4, 2, 0, 1)
    ).astype(ml_dtypes.bfloat16)  # [128 cl, 9 k, 2 ch, 2 ot, 128 ol]

    in_maps = []
    for core in range(8):
        b, half = divmod(core, 2)
        r0 = half * NROWS
        xb = x[b]  # [256, 56, 56]

        xp58 = np.zeros((CIN, 58, 58), np.float32)
        xp58[:, 1:57, 1:57] = xb
        xcf = np.ascontiguousarray(
            xp58[:, r0 : r0 + 30, :].reshape(2, 128, 30 * 58).transpose(1, 0, 2)
        ).astype(ml_dtypes.bfloat16)

        xp = np.zeros((HQ + 1, WQ + 1, CIN), np.float32)
        ylo = max(0, r0 - MARG)
        yhi = min(H, r0 + HQ + 1 - MARG)
        xhwc = xb.transpose(1, 2, 0)
        xp[ylo - (r0 - MARG) : yhi - (r0 - MARG), MARG : MARG + W, :] = xhwc[ylo:yhi]
        quad = np.stack(
            [xp[:-1, :-1], xp[:-1, 1:], xp[1:, :-1], xp[1:, 1:]], axis=2
        )  # [72, 72, 4, 256]
        xq = np.ascontiguousarray(quad.reshape(NQ, 4 * CIN)).astype(
            ml_dtypes.float8_e3m4
        )

        in_maps.append(
            {
                "xcf": xcf,
                "xq": xq,
                "woff": woff_arr,
                "boff": boff_arr,
                "wm": wm_arr,
            }
        )
    return in_maps


def unshard_output(results):
    """results: list of 8 per-core out arrays [128, 2, NSLOT] -> [B,COUT,H,W]."""
    out = np.zeros((B, COUT, H, W), np.float32)
    for core in range(8):
        b, half = divmod(core, 2)
        r0 = half * NROWS
        oc = results[core]  # [128 ol, 2 oh, NSLOT]
        oc = oc.transpose(1, 0, 2).reshape(COUT, NSLOT)[:, :NPIX]
        out[b, :, r0 : r0 + NROWS, :] = oc.reshape(COUT, NROWS, W)
    return out


def kernel(**inputs):
    nc = _CACHE.get("nc")
    if nc is None:
        nc = build_nc()
        _CACHE["nc"] = nc
    in_maps = prep_inputs(
        inputs["x"], inputs["w_off"], inputs["b_off"], inputs["w"]
    )
    res = run_bass_kernel_spmd(nc, in_maps, core_ids=list(range(8)))
    return unshard_output([r["out"] for r in res.results])
